# revision 1
# baseline (speedup 1.0000x reference)
"""Trainium2 Bass kernel for nn_AttentionBlock (GroupNorm + spatial self-attention + residual).

Full inputs in, full outputs out. Internally: data-parallel over the batch dim
(B=8) across 8 NeuronCores; each core runs an identical Bass/Tile program on
one [C=256, N=4096] image.

Per-core layout choices:
  - q,k stored [c, n] (c on partitions, 2 chunks of 128)
  - v stored transposed [n, c] (computed directly by swapping matmul operand
    roles, no on-device transpose pass)
  - attention scores computed transposed: S'[j,i] = (K^T Q)[j,i], j on
    partitions, so the AV contraction over j runs as PSUM-accumulated matmuls
  - softmax denominator: DVE accumulation of exp tiles over j-tiles, then a
    ones-vector matmul to reduce the 128 partitions; normalization applied to
    A via a broadcast tile (ones outer-product matmul)
  - all big matmuls in bf16 (1 cycle/row on PE vs 4 for fp32); accumulation is
    always fp32 in PSUM. exp(s/16) runs on ACT straight out of PSUM with the
    1/sqrt(C) folded into the activation scale; no max-subtraction (scores are
    in [-8, 7] for this distribution, exp is safe in fp32).
"""

import sys

try:
    import concourse  # noqa: F401
except ImportError:
    sys.path.insert(0, "/opt/trn_rl_repo")

import numpy as np
import ml_dtypes

import bass_rust as _bass_rust
import concourse.bacc as bacc
import concourse.tile as tile
from concourse import mybir
from concourse import bass_isa
from concourse.bass_utils import run_bass_kernel_spmd

F32 = mybir.dt.float32
BF16 = mybir.dt.bfloat16
AF = mybir.ActivationFunctionType
ALU = mybir.AluOpType
AX = mybir.AxisListType

# When True, q/k are stored fp8-e4m3 in DoubleRow [K,2,N] layout and the
# score matmuls run one fp8 DoubleRow matmul per (j-tile, i-stripe) at 2x PE
# rate (rel err ~6.7e-3 vs ~9e-4 for bf16; see numpy study).
S_FP8 = False

C = 256          # channels
N = 4096         # spatial positions (64*64)
GROUPS = 32      # groupnorm groups -> 8 channels per group
EPS = 1e-5
SCALE = C ** -0.5
NSTRIPE = 8      # stripes over the spatial dim
SW = N // NSTRIPE  # 512
NJT = N // 128   # 32 j-tiles
GSIZE = (C // GROUPS) * N  # elements per group = 32768


def _emit(nc, tc, d, parts="ABC"):
    """Emit the per-core program. d: dict of DRAM tensor handles."""
    const = tc.alloc_tile_pool(name="const", bufs=1)

    # --- x -> SBUF ---
    # The startup critical path (stats -> scale -> h -> everything) only needs
    # x at bf16 precision: bf16 quantization noise averages out over the
    # 32768-element group stats, and h is consumed in bf16 by the matmuls
    # anyway. So a half-size bf16 copy of x (host-prepared) lands first, and
    # the f32 x needed for the residual ~80us later arrives via the idle
    # gpsimd SWDGE path. bf16 DMAs are emitted before the weights (HWDGE
    # descriptor generation is a shared serial ~0.6us/DMA resource) and
    # spread across both HWDGE issuing engines.
    NSEG = 1
    SEG = N // NSEG
    xbf = []
    x_issuers = [nc.sync, nc.scalar, nc.scalar, nc.sync]
    for t in range(2):
        xb_ = const.tile([128, N], BF16, tag=f"xbf{t}", name=f"xbf{t}")
        for g in range(NSEG):
            x_issuers[t * NSEG + g].dma_start(
                xb_[:, g * SEG:(g + 1) * SEG],
                d["xbf"][t * 128:(t + 1) * 128, g * SEG:(g + 1) * SEG])
        xbf.append(xb_)

    # --- weights / params -> SBUF ---
    # All bf16 weights arrive as one packed [256, 1280] array (2 DMAs):
    # cols 0:768 = qkv_w.T, 768:1024 = proj_w.T, 1024:1280 (rows 0:128) =
    # v-bias broadcast tile. Small f32 params packed into [128, 10] (1 DMA).
    wb = []
    for t in range(2):
        w = const.tile([128, 1280], BF16, tag=f"wb{t}", name=f"wb{t}")
        nc.scalar.dma_start(w[:], d["wbig"][t * 128:(t + 1) * 128, :])
        wb.append(w)
    wq = [wb[0][:, 0:768], wb[1][:, 0:768]]
    wp = [wb[0][:, 768:1024], wb[1][:, 768:1024]]
    vbb = wb[0][:, 1024:1280]
    smalls = const.tile([128, 10], F32, tag="smalls")
    nc.scalar.dma_start(smalls[:], d["smalls"][:])
    qkvb = smalls[:, 0:4]
    projb = smalls[:, 4:6]
    nw = smalls[:, 6:8]
    nb = smalls[:, 8:10]
    gm = const.tile([128, 128], F32, tag="gm")
    nc.scalar.dma_start(gm[:], d["gm"][:])

    # f32 x tiles for the residual — allocated here, but their DMAs are
    # emitted after phase A so the (in-order) DMA resources service the
    # startup-critical bf16 x and weights first.
    xt = [const.tile([128, N], F32, tag=f"x{t}", name=f"x{t}") for t in range(2)]

    # --- phase A: groupnorm stats -> per-channel scale/bias ---
    # Fully per-chunk (chunk-major layout): chunk 0's scale/bias — and with
    # them the first h tiles and qkv matmuls — are ready before chunk 1's
    # stats have even landed.
    # pstats col for (t, kind, g) = (2t+kind)*NSEG + g; stats col = 2t+kind.
    pstats = const.tile([128, 4 * NSEG], F32, tag="pstats")
    stats = const.tile([128, 4], F32, tag="stats")
    scl = const.tile([128, 2], F32, tag="scl")
    bia = const.tile([128, 2], F32, tag="bia")
    gstats_mm = None
    with tc.tile_pool(name="scratch", bufs=2) as scr, \
         tc.tile_pool(name="pa_ps", bufs=1, space="PSUM") as pa_ps:
        for t in range(2):
            for g in range(NSEG):
                seg = xbf[t][:, g * SEG:(g + 1) * SEG]
                c0 = (2 * t + 0) * NSEG + g
                c1 = (2 * t + 1) * NSEG + g
                nc.vector.reduce_sum(pstats[:, c0:c0 + 1], seg, axis=AX.X)
                sq = scr.tile([128, SEG], F32, tag="sq")
                nc.scalar.activation(sq[:], seg, AF.Square,
                                     accum_out=pstats[:, c1:c1 + 1])
            for kind in range(2):
                tk = 2 * t + kind
                nc.vector.reduce_sum(stats[:, tk:tk + 1],
                                     pstats[:, tk * NSEG:(tk + 1) * NSEG], axis=AX.X)
            gstats = pa_ps.tile([128, 2], F32, tag=f"gstats{t}", name=f"gstats{t}")
            gstats_mm = nc.tensor.matmul(gstats[:], gm[:], stats[:, 2 * t:2 * t + 2],
                                         start=True, stop=True)
            # mex cols = [mean, ex2] for this chunk
            mex = const.tile([128, 2], F32, tag=f"mex{t}", name=f"mex{t}")
            nc.vector.tensor_scalar_mul(mex[:], gstats[:], 1.0 / GSIZE)
            mean = mex[:, 0:1]
            ex2 = mex[:, 1:2]
            var = const.tile([128, 1], F32, tag=f"var{t}", name=f"var{t}")
            std = const.tile([128, 1], F32, tag=f"std{t}", name=f"std{t}")
            rstd = const.tile([128, 1], F32, tag=f"rstd{t}", name=f"rstd{t}")
            negm2 = const.tile([128, 1], F32, tag=f"negm2{t}", name=f"negm2{t}")
            nc.vector.scalar_tensor_tensor(negm2[:], mean, -1.0, mean,
                                           op0=ALU.mult, op1=ALU.mult)
            nc.vector.scalar_tensor_tensor(var[:], ex2, EPS, negm2[:],
                                           op0=ALU.add, op1=ALU.add)
            nc.scalar.activation(std[:], var[:], AF.Sqrt)
            nc.vector.reciprocal(rstd[:], std[:])
            nc.vector.tensor_mul(scl[:, t:t + 1], nw[:, t:t + 1], rstd[:])
            mscl = const.tile([128, 1], F32, tag=f"mscl{t}", name=f"mscl{t}")
            nc.vector.tensor_mul(mscl[:], mean, scl[:, t:t + 1])
            nc.vector.tensor_sub(bia[:, t:t + 1], nb[:, t:t + 1], mscl[:])

    # f32 x for the residual — needed from the first stripe tail (~90us in);
    # issued via the idle gpsimd SWDGE path. The explicit dep on the stats
    # matmul keeps its transfers off the (in-order) DMA resources until the
    # startup-critical bf16-x/weights burst is done.
    for t in range(2):
        xdma = nc.gpsimd.dma_start(xt[t][:], d["x"][t * 128:(t + 1) * 128, :])
        _bass_rust.add_dep_helper(xdma.ins, gstats_mm.ins,
                                  reason="delay f32-x past startup DMA burst")

    # --- phase B: h = x*scl+bia (bf16), q,k ([c,n]) and vT ([n,c]) ---
    FP8 = mybir.dt.float8e4
    if S_FP8:
        # q/k in DoubleRow layout: partition p, element e <-> channel e*128+p
        qf8 = const.tile([128, 2, N], FP8, tag="qf8")
        kf8 = const.tile([128, 2, N], FP8, tag="kf8")
        qk = [qf8[:, 0, :], qf8[:, 1, :], kf8[:, 0, :], kf8[:, 1, :]]
    else:
        qk = []
        for i in range(4):  # q0,q1,k0,k1
            t_ = const.tile([128, N], BF16, tag=f"qk{i}", name=f"qk{i}")
            qk.append(t_)
    vt = []
    for j in range(NJT):
        t_ = const.tile([128, 256], BF16, tag=f"vt{j}", name=f"vt{j}")
        vt.append(t_)

    with tc.tile_pool(name="hpool", bufs=1) as hp, \
         tc.tile_pool(name="pb_ps", bufs=3, space="PSUM") as pbp, \
         tc.tile_pool(name="pv_ps", bufs=3, space="PSUM") as pvp:
        # h = x*scl + bia on ACT (per-partition scale/bias APs); pre-emit all
        # 16 tiles so production runs ahead of PE consumption.
        hs = []
        for s in range(NSTRIPE):
            sl = slice(s * SW, (s + 1) * SW)
            hts = []
            for t in range(2):
                ht = hp.tile([128, SW], BF16, tag=f"h{t}_{s}", name=f"h{t}_{s}")
                nc.scalar.activation(ht[:], xbf[t][:, sl], AF.Identity,
                                     bias=bia[:, t:t + 1], scale=scl[:, t:t + 1])
                hts.append(ht)
            hs.append(hts)
        for s in range(NSTRIPE):
            sl = slice(s * SW, (s + 1) * SW)
            hts = hs[s]
            for dt in (2, 3, 0, 1):  # k first: phase C's first matmuls need k
                ps = pbp.tile([128, SW], F32, tag="qkps", name="qkps")
                nc.tensor.matmul(ps[:], wq[0][:, dt * 128:(dt + 1) * 128], hts[0][:],
                                 start=True, stop=False)
                nc.tensor.matmul(ps[:], wq[1][:, dt * 128:(dt + 1) * 128], hts[1][:],
                                 start=False, stop=True)
                # split bias-copies q->DVE, k->ACT so neither engine exceeds
                # PE's ~27us in phase B (ACT also produces the h tiles)
                if dt < 2:
                    nc.vector.tensor_scalar_add(qk[dt][:, sl], ps[:], qkvb[:, dt:dt + 1])
                else:
                    nc.scalar.activation(qk[dt][:, sl], ps[:], AF.Identity,
                                         bias=qkvb[:, dt:dt + 1])
            for n4 in range(4):
                jt = s * 4 + n4
                psv = pvp.tile([128, 256], F32, tag="vtps", name="vtps")
                nc.tensor.matmul(psv[:], hts[0][:, n4 * 128:(n4 + 1) * 128],
                                 wq[0][:, 512:768], start=True, stop=False)
                nc.tensor.matmul(psv[:], hts[1][:, n4 * 128:(n4 + 1) * 128],
                                 wq[1][:, 512:768], start=False, stop=True)
                nc.vector.tensor_add(vt[jt][:], psv[:], vbb[:])

    # --- phase C: attention + proj + residual, per i-stripe ---
    if "C" not in parts:
        # timing variant: still write something to out so nothing is elided
        dummy = const.tile([128, 16], F32, tag="dummy")
        nc.vector.tensor_copy(dummy[:], xt[0][:, 0:16])
        nc.gpsimd.dma_start(d["out"][0:128, 0:16], dummy[:])
        const.release()
        return
    LAG = 6
    with tc.tile_pool(name="wpool", bufs=LAG + 3) as wpo, \
         tc.tile_pool(name="raccp", bufs=3) as rp, \
         tc.tile_pool(name="misc", bufs=2) as mp, \
         tc.tile_pool(name="s_ps", bufs=3, space="PSUM") as sp, \
         tc.tile_pool(name="a_ps", bufs=4, space="PSUM") as apo, \
         tc.tile_pool(name="o_ps", bufs=1, space="PSUM") as opo:

        def make_tail(ist, racc, a_ps):
            """Normalization + proj + residual for a finished stripe, split in
            three parts that are interleaved into the next stripe's matmul
            stream (the serial rsum->recip->mul chain hides behind PE work
            instead of stalling it)."""
            sl = slice(ist * SW, (ist + 1) * SW)
            st = {}

            def part1():
                # all-reduce over partitions on the (idle) gpsimd engine:
                # every partition ends up holding the softmax denominator row
                rall = mp.tile([128, 2 * SW], F32, tag="rall")
                nc.gpsimd.partition_all_reduce(rall[:], racc[:], 128,
                                               bass_isa.ReduceOp.add)
                st["rall"] = rall

            def part2a():
                rall = st["rall"]
                rsum = mp.tile([128, SW], F32, tag="rsum")
                nc.vector.tensor_add(rsum[:], rall[:, 0:SW], rall[:, SW:2 * SW])
                rinv = mp.tile([128, SW], F32, tag="rinv")
                nc.vector.reciprocal(rinv[:], rsum[:])
                st["rinv"] = rinv

            def part2b():
                a_sb = []
                for ct in range(2):
                    t_ = mp.tile([128, SW], BF16, tag=f"asb{ct}", name=f"asb{ct}")
                    nc.vector.tensor_mul(t_[:], a_ps[ct][:], st["rinv"][:])
                    a_sb.append(t_)
                st["a_sb"] = a_sb

            def part2():
                part2a()
                part2b()

            def part3():
                a_sb = st["a_sb"]
                for dt in range(2):
                    o_ps = opo.tile([128, SW], F32, tag="ops", name="ops")
                    nc.tensor.matmul(o_ps[:], wp[0][:, dt * 128:(dt + 1) * 128], a_sb[0][:],
                                     start=True, stop=False)
                    nc.tensor.matmul(o_ps[:], wp[1][:, dt * 128:(dt + 1) * 128], a_sb[1][:],
                                     start=False, stop=True)
                    o_sb = mp.tile([128, SW], F32, tag=f"osb{dt}", name=f"osb{dt}")
                    nc.vector.scalar_tensor_tensor(o_sb[:], o_ps[:], projb[:, dt:dt + 1],
                                                   xt[dt][:, sl], op0=ALU.add, op1=ALU.add)
                    nc.gpsimd.dma_start(d["out"][dt * 128:(dt + 1) * 128, sl], o_sb[:])

            return [part1, part2, part3, part2a, part2b]

        pending = None
        NPAIR = NJT // 2
        PLAG = LAG // 2
        for ist in range(NSTRIPE):
            sl = slice(ist * SW, (ist + 1) * SW)
            racc = rp.tile([128, 2 * SW], F32, tag="racc")
            a_ps = [apo.tile([128, SW], F32, tag="aps", name="aps") for _ in range(2)]
            # exp output halves of two consecutive j-tiles share one SBUF
            # tile, so the racc accumulation runs at [128,1024] granularity
            # (half the DVE per-op overhead) while PSUM stays per-jt
            # single-bank. AV matmuls run LAG steps behind production so the
            # (in-order) PE queue never head-of-line blocks on exp.
            w_pairs = {}
            for jt in range(NJT + LAG):
                if jt < NJT:
                    s_ps = sp.tile([128, SW], F32, tag="sps", name="sps")
                    if S_FP8:
                        nc.tensor.matmul(s_ps[:], kf8[:, :, jt * 128:(jt + 1) * 128],
                                         qf8[:, :, sl], start=True, stop=True,
                                         perf_mode=mybir.MatmulPerfMode.DoubleRow)
                    else:
                        nc.tensor.matmul(s_ps[:], qk[2][:, jt * 128:(jt + 1) * 128],
                                         qk[0][:, sl], start=True, stop=False)
                        nc.tensor.matmul(s_ps[:], qk[3][:, jt * 128:(jt + 1) * 128],
                                         qk[1][:, sl], start=False, stop=True)
                    p = jt // 2
                    if jt % 2 == 0:
                        w_pairs[p] = wpo.tile([128, 2 * SW], BF16, tag="wsb", name="wsb")
                    hsl = slice((jt % 2) * SW, (jt % 2 + 1) * SW)
                    nc.scalar.activation(w_pairs[p][:, hsl], s_ps[:], AF.Exp, scale=SCALE)
                    if jt % 2 == 1:
                        if p == 0:
                            nc.vector.tensor_copy(racc[:], w_pairs[p][:])
                        else:
                            nc.vector.tensor_add(racc[:], racc[:], w_pairs[p][:])
                if pending is not None:
                    if jt == 1:
                        pending[0]()
                    elif jt == 3:
                        pending[1]()
                    elif jt == 7:
                        pending[2]()
                        pending = None
                if ist == NSTRIPE - 1 and "noav" not in parts:
                    # last stripe: run the all-reduce and the fold/recip while
                    # the trailing AV matmuls still execute; only the a_sb
                    # muls and proj remain after the loop.
                    if jt == NJT:
                        last_tail = make_tail(ist, racc, a_ps)
                        last_tail[0]()          # part1: all-reduce
                        pending = None
                    elif jt == NJT + 3:
                        last_tail[3]()          # part2a: fold + reciprocal
                        pending = [last_tail[4], last_tail[2]]  # muls, proj
                if "noav" in parts:
                    continue
                if jt >= LAG:
                    j2 = jt - LAG
                    w2 = w_pairs[j2 // 2]
                    if j2 % 2 == 1:
                        del w_pairs[j2 // 2]
                    hsl = slice((j2 % 2) * SW, (j2 % 2 + 1) * SW)
                    for ct in range(2):
                        nc.tensor.matmul(a_ps[ct][:], vt[j2][:, ct * 128:(ct + 1) * 128],
                                         w2[:, hsl], start=(j2 == 0), stop=(j2 == NJT - 1))
            if "noav" in parts:
                o_sb = mp.tile([128, SW], F32, tag="osb0", name="osb0")
                nc.vector.tensor_add(o_sb[:], racc[:, 0:SW], xt[0][:, sl])
                nc.gpsimd.dma_start(d["out"][0:128, sl], o_sb[:])
                continue
            if ist < NSTRIPE - 1:
                pending = make_tail(ist, racc, a_ps)
        if pending is not None:
            for p in pending:
                p()

    const.release()


def build_program(repeat: int = 1, parts: str = "ABC"):
    nc = bacc.Bacc("TRN2", target_bir_lowering=False, debug=False, num_devices=8)
    d = {
        "x": nc.declare_dram_parameter("x", [C, N], F32, isOutput=False),
        "xbf": nc.declare_dram_parameter("xbf", [C, N], BF16, isOutput=False),
        "wbig": nc.declare_dram_parameter("wbig", [C, 1280], BF16, isOutput=False),
        "smalls": nc.declare_dram_parameter("smalls", [128, 10], F32, isOutput=False),
        "gm": nc.declare_dram_parameter("gm", [128, 128], F32, isOutput=False),
        "out": nc.declare_dram_parameter("out", [C, N], F32, isOutput=True),
    }
    with tile.TileContext(nc) as tc:
        for _ in range(repeat):
            _emit(nc, tc, d, parts)
    nc.compile()
    return nc


def make_in_maps(x, norm_w, norm_b, qkv_w, qkv_b, proj_w, proj_b):
    x = np.asarray(x, np.float32)
    B = x.shape[0]
    qkv_w = np.asarray(qkv_w, np.float32)
    qkv_b = np.asarray(qkv_b, np.float32)
    proj_w = np.asarray(proj_w, np.float32)
    proj_b = np.asarray(proj_b, np.float32)
    wbig = np.zeros((256, 1280), np.float32)
    wbig[:, 0:768] = qkv_w.T
    wbig[:, 768:1024] = proj_w.T
    wbig[0:128, 1024:1280] = np.tile(qkv_b[512:].reshape(1, 256), (128, 1))
    smalls = np.zeros((128, 10), np.float32)
    smalls[:, 0:4] = qkv_b[:512].reshape(4, 128).T
    smalls[:, 4:6] = proj_b.reshape(2, 128).T
    smalls[:, 6:8] = np.asarray(norm_w, np.float32).reshape(2, 128).T
    smalls[:, 8:10] = np.asarray(norm_b, np.float32).reshape(2, 128).T
    shared = {
        "wbig": wbig.astype(ml_dtypes.bfloat16),
        "smalls": smalls,
        "gm": (np.arange(128)[:, None] // 8 == np.arange(128)[None, :] // 8).astype(np.float32),
    }
    return [
        dict(shared,
             x=np.ascontiguousarray(x[b].reshape(C, N)),
             xbf=np.ascontiguousarray(x[b].reshape(C, N)).astype(ml_dtypes.bfloat16))
        for b in range(B)
    ]


_NC_CACHE = {}


def get_program(repeat: int = 1):
    if repeat not in _NC_CACHE:
        _NC_CACHE[repeat] = build_program(repeat)
    return _NC_CACHE[repeat]


def kernel(x, norm_w, norm_b, qkv_w, qkv_b, proj_w, proj_b):
    x = np.asarray(x, np.float32)
    B, C_, H_, W_ = x.shape
    in_maps = make_in_maps(x, norm_w, norm_b, qkv_w, qkv_b, proj_w, proj_b)
    nc = get_program()
    res = run_bass_kernel_spmd(nc, in_maps, core_ids=list(range(len(in_maps))))
    out = np.stack([np.asarray(res.results[b]["out"], np.float32) for b in range(B)])
    return out.reshape(B, C_, H_, W_)



# revision 8
# speedup vs baseline: 2.1526x; 2.1526x over previous
"""Trainium2 Bass kernel for nn_AttentionBlock (GroupNorm + spatial self-attention + residual).

Full inputs in, full outputs out. Internally: data-parallel over the batch dim
(B=8) across 8 NeuronCores; each core runs an identical Bass/Tile program on
one [C=256, N=4096] image.

Per-core design (fp8 attention pipeline):
  - q,k stored fp8-e4m3 in DoubleRow [128, 2, N] layout (channel c = r*128+p);
    score matmuls run one DR matmul per (j-tile, i-stripe) at 2x PE rate.
  - exp runs on ACT over [128, 2, 512] PSUM score PAIRS (two j-tiles per
    instruction, two PSUM banks) with the 1/sqrt(C) scale and a constant
    shift -SHIFT folded in; output is written straight to fp8 e4m3 in the
    DoubleRow layout the AV matmuls consume. The shift cancels in the
    softmax ratio and keeps exp values < 240 (e4m3 max).
  - v stored fp8 in DoubleRow pair tiles [128, 2, 256] (j = r*128+p+256*pair);
    AV contraction over j runs as PSUM-accumulated DR matmuls (2x rate).
  - softmax denominator comes from the PE too: a [128,2,1] fp8 ones-vector
    stationary against the same w tiles accumulates den = sum_j exp into a
    [1,512] PSUM row. Normalization: DVE reciprocal of that row, Pool
    partition_broadcast to 128 partitions, DVE muls. (No DVE accumulation
    chain, no partition_all_reduce.)
  - v-bias is folded into proj_b on the host (a = A@v0/den + vb =>
    out += proj@A v0/den + [proj@vb + proj_b]), so v tiles are plain
    PSUM->fp8 copies.
  - groupnorm stats on bf16 x; h = x*scl+bia produced in bf16 on DVE
    (2-scalar tensor_scalar); qkv/proj matmuls stay bf16.
"""

import sys

try:
    import concourse  # noqa: F401
except ImportError:
    sys.path.insert(0, "/opt/trn_rl_repo")

import numpy as np
import ml_dtypes

import bass_rust as _bass_rust
import concourse.bacc as bacc
import concourse.tile as tile
from concourse import mybir
from concourse import bass_isa
from concourse.bass_utils import run_bass_kernel_spmd

F32 = mybir.dt.float32
BF16 = mybir.dt.bfloat16
FP8 = mybir.dt.float8e4
AF = mybir.ActivationFunctionType
ALU = mybir.AluOpType
AX = mybir.AxisListType
DR = mybir.MatmulPerfMode.DoubleRow

C = 256          # channels
N = 4096         # spatial positions (64*64)
GROUPS = 32      # groupnorm groups -> 8 channels per group
EPS = 1e-5
SCALE = C ** -0.5
SHIFT = 3.25     # exp(s*SCALE - SHIFT): keeps fp8 w in (0, ~122]
NSTRIPE = 8      # stripes over the spatial dim
SW = N // NSTRIPE  # 512
NPAIR = N // 256   # 16 j-tile pairs
PLAG = 3         # AV/den matmuls lag this many pairs behind exp production
GSIZE = (C // GROUPS) * N  # elements per group = 32768


def _emit(nc, tc, d, parts="ABC"):
    """Emit the per-core program. d: dict of DRAM tensor handles."""
    const = tc.alloc_tile_pool(name="const", bufs=1)

    # --- x -> SBUF ---
    # Startup critical path only needs x at bf16 precision (stats noise
    # averages out over 32768-element groups). The f32 x for the residual
    # arrives later via the idle gpsimd SWDGE path.
    # smalls/gm ride the (parallel) Pool SWDGE path so the stats chain is
    # never blocked behind the serialized HWDGE descriptor stream.
    smalls = const.tile([128, 12], F32, tag="smalls")
    nc.gpsimd.dma_start(smalls[:], d["smalls"][:])
    qkvb = smalls[:, 0:4]
    projb = smalls[:, 4:6]
    nw = smalls[:, 6:8]
    nb = smalls[:, 8:10]
    nshift = smalls[:, 10:11]
    gm = const.tile([128, 128], F32, tag="gm")
    nc.gpsimd.dma_start(gm[:], d["gm"][:])

    # bf16 x arrives as 2 segment TILES per chunk so each segment's stats can
    # start the moment that segment's DMA lands (deps are per-tile).
    NSEG = 2
    SEG = N // NSEG
    xseg = [[None, None], [None, None]]
    x_issuers = [nc.sync, nc.scalar]
    for t in range(2):
        for g in range(NSEG):
            xb_ = const.tile([128, SEG], BF16, tag=f"xbf{t}_{g}", name=f"xbf{t}_{g}")
            x_issuers[g].dma_start(xb_[:],
                                   d["xbf"][t * 128:(t + 1) * 128,
                                            g * SEG:(g + 1) * SEG])
            xseg[t][g] = xb_

    def xb(t, lo, hi):
        """Slice of bf16 x chunk t, cols [lo, hi) — must stay in one segment."""
        g = lo // SEG
        assert hi <= (g + 1) * SEG
        return xseg[t][g][:, lo - g * SEG:hi - g * SEG]

    # --- weights / params -> SBUF ---
    wb = []
    for t in range(2):
        w = const.tile([128, 1024], BF16, tag=f"wb{t}", name=f"wb{t}")
        nc.scalar.dma_start(w[:], d["wbig"][t * 128:(t + 1) * 128, :])
        wb.append(w)
    wq = [wb[0][:, 0:768], wb[1][:, 0:768]]
    wp = [wb[0][:, 768:1024], wb[1][:, 768:1024]]

    # fp8 ones for the denominator matmul (DR stationary [128, 2, 1] slice;
    # tile padded to 16 cols so the Ko step stays 16-byte aligned)
    ones8 = const.tile([128, 2, 16], FP8, tag="ones8")
    nc.gpsimd.memset(ones8[:], 1.0)

    # f32 x tiles for the residual — DMAs emitted after phase A so the
    # startup-critical bf16 x and weights land first.
    xt = [const.tile([128, N], F32, tag=f"x{t}", name=f"x{t}") for t in range(2)]

    # --- phase A: groupnorm stats -> per-channel scale/bias ---
    pstats = const.tile([128, 4 * NSEG], F32, tag="pstats")
    stats = const.tile([128, 4], F32, tag="stats")
    scl = const.tile([128, 2], F32, tag="scl")
    bia = const.tile([128, 2], F32, tag="bia")
    gstats_mm = None
    with tc.tile_pool(name="scratch", bufs=2) as scr, \
         tc.tile_pool(name="pa_ps", bufs=1, space="PSUM") as pa_ps:
        for t in range(2):
            for g in range(NSEG):
                seg = xseg[t][g][:]
                c0 = (2 * t + 0) * NSEG + g
                c1 = (2 * t + 1) * NSEG + g
                # plain sum via tensor_scalar accum (4x DVE mode, output is
                # a throwaway bf16 scratch)
                tr = scr.tile([128, SEG], BF16, tag="tr")
                nc.vector.tensor_scalar(tr[:], seg, 1.0, 0.0, op0=ALU.mult,
                                        op1=ALU.add,
                                        accum_out=pstats[:, c0:c0 + 1])
                sq = scr.tile([128, SEG], BF16, tag="sq")
                nc.scalar.activation(sq[:], seg, AF.Square,
                                     accum_out=pstats[:, c1:c1 + 1])
            for kind in range(2):
                tk = 2 * t + kind
                nc.vector.reduce_sum(stats[:, tk:tk + 1],
                                     pstats[:, tk * NSEG:(tk + 1) * NSEG], axis=AX.X)
            gstats = pa_ps.tile([128, 2], F32, tag=f"gstats{t}", name=f"gstats{t}")
            gstats_mm = nc.tensor.matmul(gstats[:], gm[:], stats[:, 2 * t:2 * t + 2],
                                         start=True, stop=True)
            # mex cols = [mean, ex2] for this chunk
            mex = const.tile([128, 2], F32, tag=f"mex{t}", name=f"mex{t}")
            nc.vector.tensor_scalar_mul(mex[:], gstats[:], 1.0 / GSIZE)
            mean = mex[:, 0:1]
            ex2 = mex[:, 1:2]
            var = const.tile([128, 1], F32, tag=f"var{t}", name=f"var{t}")
            std = const.tile([128, 1], F32, tag=f"std{t}", name=f"std{t}")
            rstd = const.tile([128, 1], F32, tag=f"rstd{t}", name=f"rstd{t}")
            negm2 = const.tile([128, 1], F32, tag=f"negm2{t}", name=f"negm2{t}")
            nc.vector.scalar_tensor_tensor(negm2[:], mean, -1.0, mean,
                                           op0=ALU.mult, op1=ALU.mult)
            nc.vector.scalar_tensor_tensor(var[:], ex2, EPS, negm2[:],
                                           op0=ALU.add, op1=ALU.add)
            nc.scalar.activation(std[:], var[:], AF.Sqrt)
            nc.vector.reciprocal(rstd[:], std[:])
            nc.vector.tensor_mul(scl[:, t:t + 1], nw[:, t:t + 1], rstd[:])
            mscl = const.tile([128, 1], F32, tag=f"mscl{t}", name=f"mscl{t}")
            nc.vector.tensor_mul(mscl[:], mean, scl[:, t:t + 1])
            nc.vector.tensor_sub(bia[:, t:t + 1], nb[:, t:t + 1], mscl[:])

    # Preload the Exp activation table while phase B warms up, so the first
    # real exp doesn't pay the table switch.
    dummy_exp = const.tile([1, 4], F32, tag="dummy_exp")
    nc.scalar.activation(dummy_exp[:], smalls[0:1, 0:4], AF.Exp)

    # f32 x for the residual — needed from the first stripe tail; issued via
    # the idle gpsimd SWDGE path, delayed past the startup DMA burst.
    for t in range(2):
        xdma = nc.gpsimd.dma_start(xt[t][:], d["x"][t * 128:(t + 1) * 128, :])
        _bass_rust.add_dep_helper(xdma.ins, gstats_mm.ins,
                                  reason="delay f32-x past startup DMA burst")

    # --- phase B: h (bf16), q,k (fp8 DR [128,2,N]) and v (fp8 DR pairs) ---
    # Phase C's stripe-0 score matmuls + exp are interleaved INTO phase B as
    # the k columns they need become available (k j-tiles [4s, 4s+4) land with
    # B-stripe s), so the ACT exp stream starts ~20us earlier. Stripe 0's
    # AV/den matmuls catch up right after phase B (PSUM bank budget: during
    # overlap pbp 2 + pvp 2 + s_ps 2x2 = 8; after, s 4 + a 2 + o 1 + den 1 = 8).
    qf8 = const.tile([128, 2, N], FP8, tag="qf8")
    kf8 = const.tile([128, 2, N], FP8, tag="kf8")
    vt = []
    for p2 in range(NPAIR):
        t_ = const.tile([128, 2, 256], FP8, tag=f"vt{p2}", name=f"vt{p2}")
        vt.append(t_)

    do_c = "C" in parts

    with tc.tile_pool(name="wpool", bufs=NPAIR + 3) as wpo, \
         tc.tile_pool(name="misc", bufs=2) as mp, \
         tc.tile_pool(name="s_ps", bufs=2, space="PSUM") as sp:

        def emit_spair(ist, p2):
            """Scores for j-tile pair p2 against i-stripe ist, then exp->fp8."""
            sl = slice(ist * SW, (ist + 1) * SW)
            s_ps = sp.tile([128, 2, SW], F32, tag="sps", name="sps")
            for r in range(2):
                jt = 2 * p2 + r
                nc.tensor.matmul(s_ps[:, r, :],
                                 kf8[:, :, jt * 128:(jt + 1) * 128],
                                 qf8[:, :, sl], start=True, stop=True,
                                 perf_mode=DR)
            wd = wpo.tile([128, 2, SW], FP8, tag="wd", name="wd")
            nc.scalar.activation(wd[:], s_ps[:], AF.Exp,
                                 scale=SCALE, bias=nshift)
            return wd

        def emit_avden(p2, wd, a_ps, den):
            st_ = (p2 == 0)
            en_ = (p2 == NPAIR - 1)
            for ct in range(2):
                nc.tensor.matmul(a_ps[:, ct, :],
                                 vt[p2][:, :, ct * 128:(ct + 1) * 128],
                                 wd[:], start=st_, stop=en_, perf_mode=DR)
            nc.tensor.matmul(den[:], ones8[:, :, 0:1], wd[:],
                             start=st_, stop=en_, perf_mode=DR)

        w0 = {}
        with tc.tile_pool(name="hpool", bufs=1) as hp, \
             tc.tile_pool(name="pb_ps", bufs=2, space="PSUM") as pbp, \
             tc.tile_pool(name="pv_ps", bufs=2, space="PSUM") as pvp:
            # h = x*scl + bia on DVE (bf16 in/out -> 2x mode); pre-emit all 16
            # tiles so production runs ahead of PE consumption.
            hs = []
            for s in range(NSTRIPE):
                sl = slice(s * SW, (s + 1) * SW)
                hts = []
                for t in range(2):
                    ht = hp.tile([128, SW], BF16, tag=f"h{t}_{s}", name=f"h{t}_{s}")
                    nc.vector.tensor_scalar(ht[:], xb(t, s * SW, (s + 1) * SW),
                                            scl[:, t:t + 1], bia[:, t:t + 1],
                                            op0=ALU.mult, op1=ALU.add)
                    hts.append(ht)
                hs.append(hts)
            for s in range(NSTRIPE):
                sl = slice(s * SW, (s + 1) * SW)
                hts = hs[s]
                for dt in (2, 3, 0, 1):  # k first: the overlapped scores need k
                    ps = pbp.tile([128, SW], F32, tag="qkps", name="qkps")
                    nc.tensor.matmul(ps[:], wq[0][:, dt * 128:(dt + 1) * 128], hts[0][:],
                                     start=True, stop=False)
                    nc.tensor.matmul(ps[:], wq[1][:, dt * 128:(dt + 1) * 128], hts[1][:],
                                     start=False, stop=True)
                    # bias-add + fp8 cast into DR layout, all on DVE (ACT is
                    # the kernel-wide bottleneck; vt copies below stay on ACT)
                    dst = (kf8 if dt >= 2 else qf8)[:, dt % 2, sl]
                    nc.vector.tensor_scalar_add(dst, ps[:], qkvb[:, dt:dt + 1])
                for n4 in range(4):
                    jt = s * 4 + n4
                    psv = pvp.tile([128, 256], F32, tag="vtps", name="vtps")
                    nc.tensor.matmul(psv[:], hts[0][:, n4 * 128:(n4 + 1) * 128],
                                     wq[0][:, 512:768], start=True, stop=False)
                    nc.tensor.matmul(psv[:], hts[1][:, n4 * 128:(n4 + 1) * 128],
                                     wq[1][:, 512:768], start=False, stop=True)
                    # v-bias folded into proj_b host-side: plain fp8 cast
                    # (Identity on ACT, same table set as Exp)
                    dst = vt[jt // 2][:, jt % 2, :]
                    nc.scalar.activation(dst, psv[:], AF.Identity)
                if do_c and s >= 1:
                    for p2 in (2 * (s - 1), 2 * s - 1):
                        w0[p2] = emit_spair(0, p2)

        if not do_c:
            dummy = const.tile([128, 16], F32, tag="dummy")
            nc.vector.tensor_copy(dummy[:], xt[0][:, 0:16])
            nc.gpsimd.dma_start(d["out"][0:128, 0:16], dummy[:])
            const.release()
            return

        # --- phase C: attention + proj + residual, per i-stripe ---
        with tc.tile_pool(name="a_ps", bufs=1, space="PSUM") as apo, \
             tc.tile_pool(name="o_ps", bufs=1, space="PSUM") as opo, \
             tc.tile_pool(name="d_ps", bufs=1, space="PSUM") as dpo:

            def make_tail(ist, den, a_ps):
                """Normalization + proj + residual for a finished stripe,
                split into parts interleaved into the next stripe's stream."""
                sl = slice(ist * SW, (ist + 1) * SW)
                st = {}

                def part1():
                    rinv = mp.tile([1, SW], F32, tag="rinv")
                    nc.vector.reciprocal(rinv[:], den[:])
                    st["rinv"] = rinv

                def part2a():
                    rbc = mp.tile([128, SW], F32, tag="rbc")
                    nc.gpsimd.partition_broadcast(rbc[:], st["rinv"][:])
                    st["rbc"] = rbc

                def part2b():
                    a_sb = []
                    for ct in range(2):
                        t_ = mp.tile([128, SW], BF16, tag=f"asb{ct}", name=f"asb{ct}")
                        nc.vector.tensor_mul(t_[:], a_ps[:, ct, :], st["rbc"][:])
                        a_sb.append(t_)
                    st["a_sb"] = a_sb

                def part3():
                    a_sb = st["a_sb"]
                    for dt in range(2):
                        o_ps = opo.tile([128, SW], F32, tag="ops", name="ops")
                        nc.tensor.matmul(o_ps[:], wp[0][:, dt * 128:(dt + 1) * 128], a_sb[0][:],
                                         start=True, stop=False)
                        nc.tensor.matmul(o_ps[:], wp[1][:, dt * 128:(dt + 1) * 128], a_sb[1][:],
                                         start=False, stop=True)
                        o_sb = mp.tile([128, SW], F32, tag=f"osb{dt}", name=f"osb{dt}")
                        nc.vector.scalar_tensor_tensor(o_sb[:], o_ps[:], projb[:, dt:dt + 1],
                                                       xt[dt][:, sl], op0=ALU.add, op1=ALU.add)
                        nc.gpsimd.dma_start(d["out"][dt * 128:(dt + 1) * 128, sl], o_sb[:])

                return [part1, part2a, part2b, part3]

            # stripe 0: last two score pairs, then the AV/den catch-up burst
            a_ps = apo.tile([128, 2, SW], F32, tag="aps", name="aps")
            den = dpo.tile([1, SW], F32, tag="den", name="den")
            for p2 in (NPAIR - 2, NPAIR - 1):
                w0[p2] = emit_spair(0, p2)
            for p2 in range(NPAIR):
                emit_avden(p2, w0.pop(p2), a_ps, den)
            pending = make_tail(0, den, a_ps)

            for ist in range(1, NSTRIPE):
                a_ps = apo.tile([128, 2, SW], F32, tag="aps", name="aps")
                den = dpo.tile([1, SW], F32, tag="den", name="den")
                w_tiles = {}
                for p2 in range(NPAIR + PLAG):
                    if p2 < NPAIR:
                        w_tiles[p2] = emit_spair(ist, p2)
                    if pending is not None:
                        if p2 == 0:
                            pending[0]()          # reciprocal of den row
                        elif p2 == 1:
                            pending[1]()          # Pool broadcast
                        elif p2 == 2:
                            pending[2]()          # a_sb muls (releases prev a_ps)
                        elif p2 == 4:
                            pending[3]()          # proj + residual + DMA
                            pending = None
                    if p2 >= PLAG:
                        emit_avden(p2 - PLAG, w_tiles.pop(p2 - PLAG), a_ps, den)
                pending = make_tail(ist, den, a_ps)
            if pending is not None:
                for p in pending:
                    p()

    const.release()


def build_program(repeat: int = 1, parts: str = "ABC"):
    nc = bacc.Bacc("TRN2", target_bir_lowering=False, debug=False, num_devices=8)
    d = {
        "x": nc.declare_dram_parameter("x", [C, N], F32, isOutput=False),
        "xbf": nc.declare_dram_parameter("xbf", [C, N], BF16, isOutput=False),
        "wbig": nc.declare_dram_parameter("wbig", [C, 1024], BF16, isOutput=False),
        "smalls": nc.declare_dram_parameter("smalls", [128, 12], F32, isOutput=False),
        "gm": nc.declare_dram_parameter("gm", [128, 128], F32, isOutput=False),
        "out": nc.declare_dram_parameter("out", [C, N], F32, isOutput=True),
    }
    with tile.TileContext(nc) as tc:
        for _ in range(repeat):
            _emit(nc, tc, d, parts)
    nc.compile()
    return nc


def make_in_maps(x, norm_w, norm_b, qkv_w, qkv_b, proj_w, proj_b):
    x = np.asarray(x, np.float32)
    B = x.shape[0]
    qkv_w = np.asarray(qkv_w, np.float32)
    qkv_b = np.asarray(qkv_b, np.float32)
    proj_w = np.asarray(proj_w, np.float32)
    proj_b = np.asarray(proj_b, np.float32)
    # v-bias folded into proj bias: out = proj@(A v0/den) + (proj@vb + proj_b)
    projb_eff = proj_b + proj_w @ qkv_b[512:]
    wbig = np.zeros((256, 1024), np.float32)
    wbig[:, 0:768] = qkv_w.T
    wbig[:, 768:1024] = proj_w.T
    smalls = np.zeros((128, 12), np.float32)
    smalls[:, 10] = -SHIFT
    smalls[:, 0:4] = qkv_b[:512].reshape(4, 128).T
    smalls[:, 4:6] = projb_eff.reshape(2, 128).T
    smalls[:, 6:8] = np.asarray(norm_w, np.float32).reshape(2, 128).T
    smalls[:, 8:10] = np.asarray(norm_b, np.float32).reshape(2, 128).T
    shared = {
        "wbig": wbig.astype(ml_dtypes.bfloat16),
        "smalls": smalls,
        "gm": (np.arange(128)[:, None] // 8 == np.arange(128)[None, :] // 8).astype(np.float32),
    }
    return [
        dict(shared,
             x=np.ascontiguousarray(x[b].reshape(C, N)),
             xbf=np.ascontiguousarray(x[b].reshape(C, N)).astype(ml_dtypes.bfloat16))
        for b in range(B)
    ]


_NC_CACHE = {}


def get_program(repeat: int = 1):
    if repeat not in _NC_CACHE:
        _NC_CACHE[repeat] = build_program(repeat)
    return _NC_CACHE[repeat]


def kernel(x, norm_w, norm_b, qkv_w, qkv_b, proj_w, proj_b):
    x = np.asarray(x, np.float32)
    B, C_, H_, W_ = x.shape
    in_maps = make_in_maps(x, norm_w, norm_b, qkv_w, qkv_b, proj_w, proj_b)
    nc = get_program()
    res = run_bass_kernel_spmd(nc, in_maps, core_ids=list(range(len(in_maps))))
    out = np.stack([np.asarray(res.results[b]["out"], np.float32) for b in range(B)])
    return out.reshape(B, C_, H_, W_)


# revision 11
# speedup vs baseline: 2.3113x; 1.0737x over previous
"""Trainium2 Bass kernel for nn_AttentionBlock (GroupNorm + spatial self-attention + residual).

Full inputs in, full outputs out. Internally: data-parallel over the batch dim
(B=8) across 8 NeuronCores; each core runs an identical Bass/Tile program on
one [C=256, N=4096] image.

Per-core design (fp8 attention pipeline):
  - q,k stored fp8-e4m3 in DoubleRow [128, 2, N] layout (channel c = r*128+p);
    score matmuls run one DR matmul per (j-tile, i-stripe) at 2x PE rate.
  - exp runs on ACT over [128, 2, 512] PSUM score PAIRS (two j-tiles per
    instruction, two PSUM banks) with the 1/sqrt(C) scale and a constant
    shift -SHIFT folded in; output is written straight to fp8 e4m3 in the
    DoubleRow layout the AV matmuls consume. The shift cancels in the
    softmax ratio and keeps exp values < 240 (e4m3 max).
  - v stored fp8 in DoubleRow pair tiles [128, 2, 256] (j = r*128+p+256*pair);
    AV contraction over j runs as PSUM-accumulated DR matmuls (2x rate).
  - softmax denominator comes from the PE too: a [128,2,1] fp8 ones-vector
    stationary against the same w tiles accumulates den = sum_j exp into a
    [1,512] PSUM row. Normalization: DVE reciprocal of that row, Pool
    partition_broadcast to 128 partitions, DVE muls. (No DVE accumulation
    chain, no partition_all_reduce.)
  - v-bias is folded into proj_b on the host (a = A@v0/den + vb =>
    out += proj@A v0/den + [proj@vb + proj_b]), so v tiles are plain
    PSUM->fp8 copies.
  - groupnorm stats on bf16 x; h = x*scl+bia produced in bf16 on DVE
    (2-scalar tensor_scalar); qkv/proj matmuls stay bf16.
"""

import sys

try:
    import concourse  # noqa: F401
except ImportError:
    sys.path.insert(0, "/opt/trn_rl_repo")

import numpy as np
import ml_dtypes

import bass_rust as _bass_rust
import concourse.bacc as bacc
import concourse.tile as tile
from concourse import mybir
from concourse import bass_isa
from concourse.bass_utils import run_bass_kernel_spmd

F32 = mybir.dt.float32
BF16 = mybir.dt.bfloat16
FP8 = mybir.dt.float8e4
AF = mybir.ActivationFunctionType
ALU = mybir.AluOpType
AX = mybir.AxisListType
DR = mybir.MatmulPerfMode.DoubleRow

C = 256          # channels
N = 4096         # spatial positions (64*64)
GROUPS = 32      # groupnorm groups -> 8 channels per group
EPS = 1e-5
SCALE = C ** -0.5
SHIFT = 3.25     # exp(s*SCALE - SHIFT): keeps fp8 w in (0, ~122]
NSTRIPE = 8      # stripes over the spatial dim
SW = N // NSTRIPE  # 512
NPAIR = N // 256   # 16 j-tile pairs
PLAG = 3         # AV/den matmuls lag this many pairs behind exp production
GSIZE = (C // GROUPS) * N  # elements per group = 32768


def _emit(nc, tc, d, parts="ABC"):
    """Emit the per-core program. d: dict of DRAM tensor handles."""
    const = tc.alloc_tile_pool(name="const", bufs=1)

    # --- x -> SBUF ---
    # Startup critical path only needs x at bf16 precision (stats noise
    # averages out over 32768-element groups). The f32 x for the residual
    # arrives later via the idle gpsimd SWDGE path.
    # smalls/gm ride the (parallel) Pool SWDGE path so the stats chain is
    # never blocked behind the serialized HWDGE descriptor stream.
    smalls = const.tile([128, 12], F32, tag="smalls")
    nc.gpsimd.dma_start(smalls[:], d["smalls"][:])
    qkvb = smalls[:, 0:4]
    projb = smalls[:, 4:6]
    nw = smalls[:, 6:8]
    nb = smalls[:, 8:10]
    nshift = smalls[:, 10:11]
    gm = const.tile([128, 128], F32, tag="gm")
    nc.gpsimd.dma_start(gm[:], d["gm"][:])

    # bf16 x arrives as 2 segment TILES per chunk so each segment's stats can
    # start the moment that segment's DMA lands (deps are per-tile).
    NSEG = 2
    SEG = N // NSEG
    xseg = [[None, None], [None, None]]
    x_issuers = [nc.sync, nc.scalar]
    for t in range(2):
        for g in range(NSEG):
            xb_ = const.tile([128, SEG], BF16, tag=f"xbf{t}_{g}", name=f"xbf{t}_{g}")
            x_issuers[g].dma_start(xb_[:],
                                   d["xbf"][t * 128:(t + 1) * 128,
                                            g * SEG:(g + 1) * SEG])
            xseg[t][g] = xb_

    def xb(t, lo, hi):
        """Slice of bf16 x chunk t, cols [lo, hi) — must stay in one segment."""
        g = lo // SEG
        assert hi <= (g + 1) * SEG
        return xseg[t][g][:, lo - g * SEG:hi - g * SEG]

    # --- weights -> SBUF ---
    # All matmul weights ship as one fp8 tensor in DoubleRow layout:
    # dram row p*2+r <-> contraction channel c = r*128 + p. Cols 0:512 are
    # the q,k output blocks of qkv_w.T, 512:768 the v block, 768:1024 proj_w.T.
    w8 = const.tile([128, 2, 1024], FP8, tag="w8")
    nc.scalar.dma_start(w8[:], d["w8"][:])

    # fp8 ones for the denominator matmul (DR stationary [128, 2, 1] slice;
    # tile padded to 16 cols so the Ko step stays 16-byte aligned)
    ones8 = const.tile([128, 2, 16], FP8, tag="ones8")
    nc.gpsimd.memset(ones8[:], 1.0)

    # f32 x tiles for the residual — DMAs emitted after phase A so the
    # startup-critical bf16 x and weights land first.
    xt = [const.tile([128, N], F32, tag=f"x{t}", name=f"x{t}") for t in range(2)]

    # --- phase A: groupnorm stats -> per-channel scale/bias ---
    pstats = const.tile([128, 4 * NSEG], F32, tag="pstats")
    stats = const.tile([128, 4], F32, tag="stats")
    scl = const.tile([128, 2], F32, tag="scl")
    bia = const.tile([128, 2], F32, tag="bia")
    gstats_mm = None
    with tc.tile_pool(name="scratch", bufs=2) as scr, \
         tc.tile_pool(name="pa_ps", bufs=1, space="PSUM") as pa_ps:
        for t in range(2):
            for g in range(NSEG):
                seg = xseg[t][g][:]
                c0 = (2 * t + 0) * NSEG + g
                c1 = (2 * t + 1) * NSEG + g
                # plain sum via tensor_scalar accum (4x DVE mode, output is
                # a throwaway bf16 scratch)
                tr = scr.tile([128, SEG], BF16, tag="tr")
                nc.vector.tensor_scalar(tr[:], seg, 1.0, 0.0, op0=ALU.mult,
                                        op1=ALU.add,
                                        accum_out=pstats[:, c0:c0 + 1])
                sq = scr.tile([128, SEG], BF16, tag="sq")
                nc.scalar.activation(sq[:], seg, AF.Square,
                                     accum_out=pstats[:, c1:c1 + 1])
            for kind in range(2):
                tk = 2 * t + kind
                nc.vector.reduce_sum(stats[:, tk:tk + 1],
                                     pstats[:, tk * NSEG:(tk + 1) * NSEG], axis=AX.X)
            gstats = pa_ps.tile([128, 2], F32, tag=f"gstats{t}", name=f"gstats{t}")
            gstats_mm = nc.tensor.matmul(gstats[:], gm[:], stats[:, 2 * t:2 * t + 2],
                                         start=True, stop=True)
            # mex cols = [mean, ex2] for this chunk
            mex = const.tile([128, 2], F32, tag=f"mex{t}", name=f"mex{t}")
            nc.vector.tensor_scalar_mul(mex[:], gstats[:], 1.0 / GSIZE)
            mean = mex[:, 0:1]
            ex2 = mex[:, 1:2]
            var = const.tile([128, 1], F32, tag=f"var{t}", name=f"var{t}")
            std = const.tile([128, 1], F32, tag=f"std{t}", name=f"std{t}")
            rstd = const.tile([128, 1], F32, tag=f"rstd{t}", name=f"rstd{t}")
            negm2 = const.tile([128, 1], F32, tag=f"negm2{t}", name=f"negm2{t}")
            nc.vector.scalar_tensor_tensor(negm2[:], mean, -1.0, mean,
                                           op0=ALU.mult, op1=ALU.mult)
            nc.vector.scalar_tensor_tensor(var[:], ex2, EPS, negm2[:],
                                           op0=ALU.add, op1=ALU.add)
            nc.scalar.activation(std[:], var[:], AF.Sqrt)
            nc.vector.reciprocal(rstd[:], std[:])
            nc.vector.tensor_mul(scl[:, t:t + 1], nw[:, t:t + 1], rstd[:])
            mscl = const.tile([128, 1], F32, tag=f"mscl{t}", name=f"mscl{t}")
            nc.vector.tensor_mul(mscl[:], mean, scl[:, t:t + 1])
            nc.vector.tensor_sub(bia[:, t:t + 1], nb[:, t:t + 1], mscl[:])

    # Preload the Exp activation table while phase B warms up, so the first
    # real exp doesn't pay the table switch.
    dummy_exp = const.tile([1, 4], F32, tag="dummy_exp")
    nc.scalar.activation(dummy_exp[:], smalls[0:1, 0:4], AF.Exp)


    # --- phase B: h (bf16), q,k (fp8 DR [128,2,N]) and v (fp8 DR pairs) ---
    # Phase C's stripe-0 score matmuls + exp are interleaved INTO phase B as
    # the k columns they need become available (k j-tiles [4s, 4s+4) land with
    # B-stripe s), so the ACT exp stream starts ~20us earlier. Stripe 0's
    # AV/den matmuls catch up right after phase B (PSUM bank budget: during
    # overlap pbp 2 + pvp 2 + s_ps 2x2 = 8; after, s 4 + a 2 + o 1 + den 1 = 8).
    qf8 = const.tile([128, 2, N], FP8, tag="qf8")
    kf8 = const.tile([128, 2, N], FP8, tag="kf8")
    vt = []
    for p2 in range(NPAIR):
        t_ = const.tile([128, 2, 256], FP8, tag=f"vt{p2}", name=f"vt{p2}")
        vt.append(t_)

    do_c = "C" in parts

    with tc.tile_pool(name="wpool", bufs=NPAIR + 3) as wpo, \
         tc.tile_pool(name="misc", bufs=2) as mp, \
         tc.tile_pool(name="s_ps", bufs=2, space="PSUM") as sp:

        def emit_spair(ist, p2):
            """Scores for j-tile pair p2 against i-stripe ist, then exp->fp8."""
            sl = slice(ist * SW, (ist + 1) * SW)
            s_ps = sp.tile([128, 2, SW], F32, tag="sps", name="sps")
            for r in range(2):
                jt = 2 * p2 + r
                nc.tensor.matmul(s_ps[:, r, :],
                                 kf8[:, :, jt * 128:(jt + 1) * 128],
                                 qf8[:, :, sl], start=True, stop=True,
                                 perf_mode=DR)
            wd = wpo.tile([128, 2, SW], FP8, tag="wd", name="wd")
            nc.scalar.activation(wd[:], s_ps[:], AF.Exp,
                                 scale=SCALE, bias=nshift)
            return wd

        def emit_avden(p2, wd, a_ps, den):
            st_ = (p2 == 0)
            en_ = (p2 == NPAIR - 1)
            for ct in range(2):
                nc.tensor.matmul(a_ps[:, ct, :],
                                 vt[p2][:, :, ct * 128:(ct + 1) * 128],
                                 wd[:], start=st_, stop=en_, perf_mode=DR)
            nc.tensor.matmul(den[:], ones8[:, :, 0:1], wd[:],
                             start=st_, stop=en_, perf_mode=DR)

        w0 = {}
        h_dr = const.tile([128, 2, N], FP8, tag="h_dr")
        with tc.tile_pool(name="pb_ps", bufs=2, space="PSUM") as pbp, \
             tc.tile_pool(name="pv_ps", bufs=2, space="PSUM") as pvp:
            # h = x*scl + bia written straight to the fp8 DR layout the qkv
            # matmuls consume. Chunk-0 slices first (their scale/bias is ready
            # earlier); the first stripes ride DVE so the opening qkv matmuls
            # unblock as soon as chunk-1 stats land, the rest ride Pool.
            for t in range(2):
                for s in range(NSTRIPE):
                    sl = slice(s * SW, (s + 1) * SW)
                    eng = nc.vector if s < 2 else nc.gpsimd
                    eng.tensor_scalar(h_dr[:, t, sl], xb(t, s * SW, (s + 1) * SW),
                                      scl[:, t:t + 1], bia[:, t:t + 1],
                                      op0=ALU.mult, op1=ALU.add)
            # f32 x for the residual — needed from the first stripe tail;
            # issued via the gpsimd SWDGE path AFTER the h ops so its (long)
            # descriptor generation doesn't block them, and delayed past the
            # startup DMA burst.
            for t in range(2):
                xdma = nc.gpsimd.dma_start(xt[t][:], d["x"][t * 128:(t + 1) * 128, :])
                _bass_rust.add_dep_helper(xdma.ins, gstats_mm.ins,
                                          reason="delay f32-x past startup DMA burst")
            for s in range(NSTRIPE):
                sl = slice(s * SW, (s + 1) * SW)
                for dt in (2, 3, 0, 1):  # k first: the overlapped scores need k
                    ps = pbp.tile([128, SW], F32, tag="qkps", name="qkps")
                    nc.tensor.matmul(ps[:], w8[:, :, dt * 128:(dt + 1) * 128],
                                     h_dr[:, :, sl], start=True, stop=True,
                                     perf_mode=DR)
                    # bias-add + fp8 cast into DR layout, all on DVE
                    dst = (kf8 if dt >= 2 else qf8)[:, dt % 2, sl]
                    nc.vector.tensor_scalar_add(dst, ps[:], qkvb[:, dt:dt + 1])
                for n4 in range(4):
                    jt = s * 4 + n4
                    psv = pvp.tile([128, 256], F32, tag="vtps", name="vtps")
                    nc.tensor.matmul(psv[:], h_dr[:, :, jt * 128:(jt + 1) * 128],
                                     w8[:, :, 512:768], start=True, stop=True,
                                     perf_mode=DR)
                    # v-bias folded into proj_b host-side: plain fp8 cast
                    # (Pool cannot read PSUM); split DVE/ACT to balance the
                    # phase-B pace
                    dst = vt[jt // 2][:, jt % 2, :]
                    if n4 % 2 == 0:
                        nc.vector.tensor_copy(dst, psv[:])
                    else:
                        nc.scalar.activation(dst, psv[:], AF.Identity)
                if do_c and s >= 1:
                    for p2 in (2 * (s - 1), 2 * s - 1):
                        w0[p2] = emit_spair(0, p2)

        if not do_c:
            dummy = const.tile([128, 16], F32, tag="dummy")
            nc.vector.tensor_copy(dummy[:], xt[0][:, 0:16])
            nc.gpsimd.dma_start(d["out"][0:128, 0:16], dummy[:])
            const.release()
            return

        # --- phase C: attention + proj + residual, per i-stripe ---
        with tc.tile_pool(name="a_ps", bufs=1, space="PSUM") as apo, \
             tc.tile_pool(name="o_ps", bufs=1, space="PSUM") as opo, \
             tc.tile_pool(name="d_ps", bufs=1, space="PSUM") as dpo:

            def make_tail(ist, den, a_ps):
                """Normalization + proj + residual for a finished stripe,
                split into parts interleaved into the next stripe's stream."""
                sl = slice(ist * SW, (ist + 1) * SW)
                st = {}

                def part1():
                    rinv = mp.tile([1, SW], F32, tag="rinv")
                    nc.vector.reciprocal(rinv[:], den[:])
                    st["rinv"] = rinv

                def part2a():
                    rbc = mp.tile([128, SW], F32, tag="rbc")
                    nc.gpsimd.partition_broadcast(rbc[:], st["rinv"][:])
                    st["rbc"] = rbc

                def part2b():
                    a_dr = mp.tile([128, 2, SW], FP8, tag="adr", name="adr")
                    for ct in range(2):
                        nc.vector.tensor_mul(a_dr[:, ct, :], a_ps[:, ct, :], st["rbc"][:])
                    st["a_dr"] = a_dr

                def part3():
                    a_dr = st["a_dr"]
                    for dt in range(2):
                        o_ps = opo.tile([128, SW], F32, tag="ops", name="ops")
                        nc.tensor.matmul(o_ps[:], w8[:, :, 768 + dt * 128:768 + (dt + 1) * 128],
                                         a_dr[:], start=True, stop=True, perf_mode=DR)
                        o_sb = mp.tile([128, SW], F32, tag=f"osb{dt}", name=f"osb{dt}")
                        nc.vector.scalar_tensor_tensor(o_sb[:], o_ps[:], projb[:, dt:dt + 1],
                                                       xt[dt][:, sl], op0=ALU.add, op1=ALU.add)
                        nc.gpsimd.dma_start(d["out"][dt * 128:(dt + 1) * 128, sl], o_sb[:])

                return [part1, part2a, part2b, part3]

            # stripe 0: last two score pairs, then the AV/den catch-up burst
            a_ps = apo.tile([128, 2, SW], F32, tag="aps", name="aps")
            den = dpo.tile([1, SW], F32, tag="den", name="den")
            for p2 in (NPAIR - 2, NPAIR - 1):
                w0[p2] = emit_spair(0, p2)
            for p2 in range(NPAIR):
                emit_avden(p2, w0.pop(p2), a_ps, den)
            pending = make_tail(0, den, a_ps)

            for ist in range(1, NSTRIPE):
                a_ps = apo.tile([128, 2, SW], F32, tag="aps", name="aps")
                den = dpo.tile([1, SW], F32, tag="den", name="den")
                w_tiles = {}
                for p2 in range(NPAIR + PLAG):
                    if p2 < NPAIR:
                        w_tiles[p2] = emit_spair(ist, p2)
                    if pending is not None:
                        if p2 == 0:
                            pending[0]()          # reciprocal of den row
                        elif p2 == 1:
                            pending[1]()          # Pool broadcast
                        elif p2 == 2:
                            pending[2]()          # a_sb muls (releases prev a_ps)
                        elif p2 == 4:
                            pending[3]()          # proj + residual + DMA
                            pending = None
                    if p2 >= PLAG:
                        emit_avden(p2 - PLAG, w_tiles.pop(p2 - PLAG), a_ps, den)
                pending = make_tail(ist, den, a_ps)
            if pending is not None:
                for p in pending:
                    p()

    const.release()


def build_program(repeat: int = 1, parts: str = "ABC"):
    nc = bacc.Bacc("TRN2", target_bir_lowering=False, debug=False, num_devices=8)
    d = {
        "x": nc.declare_dram_parameter("x", [C, N], F32, isOutput=False),
        "xbf": nc.declare_dram_parameter("xbf", [C, N], BF16, isOutput=False),
        "w8": nc.declare_dram_parameter("w8", [C, 1024], FP8, isOutput=False),
        "smalls": nc.declare_dram_parameter("smalls", [128, 12], F32, isOutput=False),
        "gm": nc.declare_dram_parameter("gm", [128, 128], F32, isOutput=False),
        "out": nc.declare_dram_parameter("out", [C, N], F32, isOutput=True),
    }
    with tile.TileContext(nc) as tc:
        for _ in range(repeat):
            _emit(nc, tc, d, parts)
    nc.compile()
    return nc


def make_in_maps(x, norm_w, norm_b, qkv_w, qkv_b, proj_w, proj_b):
    x = np.asarray(x, np.float32)
    B = x.shape[0]
    qkv_w = np.asarray(qkv_w, np.float32)
    qkv_b = np.asarray(qkv_b, np.float32)
    proj_w = np.asarray(proj_w, np.float32)
    proj_b = np.asarray(proj_b, np.float32)
    # v-bias folded into proj bias: out = proj@(A v0/den) + (proj@vb + proj_b)
    projb_eff = proj_b + proj_w @ qkv_b[512:]
    # fp8 weights in DoubleRow layout: dram row p*2+r <-> channel r*128+p
    wflat = np.zeros((256, 1024), np.float32)
    wflat[:, 0:768] = qkv_w.T
    wflat[:, 768:1024] = proj_w.T
    w8 = np.zeros((256, 1024), np.float32)
    p = np.arange(128)
    for r in range(2):
        w8[p * 2 + r, :] = wflat[r * 128 + p, :]
    w8 = np.clip(w8, -240, 240)
    smalls = np.zeros((128, 12), np.float32)
    smalls[:, 10] = -SHIFT
    smalls[:, 0:4] = qkv_b[:512].reshape(4, 128).T
    smalls[:, 4:6] = projb_eff.reshape(2, 128).T
    smalls[:, 6:8] = np.asarray(norm_w, np.float32).reshape(2, 128).T
    smalls[:, 8:10] = np.asarray(norm_b, np.float32).reshape(2, 128).T
    shared = {
        "w8": w8.astype(ml_dtypes.float8_e4m3fn),
        "smalls": smalls,
        "gm": (np.arange(128)[:, None] // 8 == np.arange(128)[None, :] // 8).astype(np.float32),
    }
    return [
        dict(shared,
             x=np.ascontiguousarray(x[b].reshape(C, N)),
             xbf=np.ascontiguousarray(x[b].reshape(C, N)).astype(ml_dtypes.bfloat16))
        for b in range(B)
    ]


_NC_CACHE = {}


def get_program(repeat: int = 1):
    if repeat not in _NC_CACHE:
        _NC_CACHE[repeat] = build_program(repeat)
    return _NC_CACHE[repeat]


def kernel(x, norm_w, norm_b, qkv_w, qkv_b, proj_w, proj_b):
    x = np.asarray(x, np.float32)
    B, C_, H_, W_ = x.shape
    in_maps = make_in_maps(x, norm_w, norm_b, qkv_w, qkv_b, proj_w, proj_b)
    nc = get_program()
    res = run_bass_kernel_spmd(nc, in_maps, core_ids=list(range(len(in_maps))))
    out = np.stack([np.asarray(res.results[b]["out"], np.float32) for b in range(B)])
    return out.reshape(B, C_, H_, W_)
